# revision 19
# baseline (speedup 1.0000x reference)
"""AutoCorrelation (Autoformer-style) TRN2 Bass kernel.

Contract: kernel(**inputs) with full unsharded inputs
  queries, keys, values: [B=16, L=2048, H=16, E=64] float32
  attn_mask: [1] bool (unused)
returns full output [B, L, H, E] float32.

Math (per (b,h,e) row, independent -> batch-sharded over 8 NeuronCores):
  1. rfft over L of q and k          (DFT as fp16 matmuls with cos/sin tables)
  2. band-pass keep bins 1..L/2-1    (tables omit Nyquist; DC of product zeroed)
  3. corr = irfft(q_fft * conj(k_fft))
  4. top-15 lags per row             (VectorE max8/max_index/match_replace)
  5. softmax weights                 (ScalarE exp)
  6. out[l] = sum_i w_i v[(l+d_i)%L] computed in frequency domain:
       s[t] = sum_i w_i delta(t=d_i) (GPSIMD local_scatter -> PE transpose)
       out  = irfft(V * conj(S))     (matmuls; DC exact via tables,
                                      Nyquist via rank-1 matmul correction)

Per core: 2 batches x 16 heads x 64 chan = 2048 rows of length L=2048,
processed in 8 groups of 256 rows (4 heads).
"""

import math
import os
import numpy as np

B, L, H, E = 16, 2048, 16, 64
NCORES = 8
BPC = B // NCORES            # batches per core
F = L // 2                   # frequencies k = 0..F-1 (Nyquist handled separately)
TOPK = 15
R = 256                      # rows per group
HPG = R // E                 # heads per group
GPB = H // HPG               # groups per batch
GROUPS = BPC * GPB           # groups per core
KC = F // 128                # k chunks (8)
TC = L // 128                # t chunks (16)
NEG = -3.0e38

_state = {}


# ---------------------------------------------------------------- device code

def build_nc(groups=GROUPS):
    import concourse.bass as bass
    import concourse.bacc as bacc
    import concourse.mybir as mybir
    from concourse.tile import TileContext

    FP16 = mybir.dt.float16
    F32 = mybir.dt.float32
    U16 = mybir.dt.uint16
    I16 = mybir.dt.int16
    AX = mybir.AxisListType.X
    MUL = mybir.AluOpType.mult

    nc = bacc.Bacc("TRN2", target_bir_lowering=False, debug=False)
    qh_h = nc.declare_dram_parameter("qh", [BPC, L, H, E], FP16, isOutput=False)
    ql_h = nc.declare_dram_parameter("ql", [BPC, L, H, E], FP16, isOutput=False)
    kh_h = nc.declare_dram_parameter("kh", [BPC, L, H, E], FP16, isOutput=False)
    kl_h = nc.declare_dram_parameter("kl", [BPC, L, H, E], FP16, isOutput=False)
    v_h = nc.declare_dram_parameter("v", [BPC, L, H, E], FP16, isOutput=False)
    cosFh_h = nc.declare_dram_parameter("cosFh", [L, F], FP16, isOutput=False)
    sinFh_h = nc.declare_dram_parameter("sinFh", [L, F], FP16, isOutput=False)
    cosFl_h = nc.declare_dram_parameter("cosFl", [L, F], FP16, isOutput=False)
    sinFl_h = nc.declare_dram_parameter("sinFl", [L, F], FP16, isOutput=False)
    cosIh_h = nc.declare_dram_parameter("cosIh", [F, L], FP16, isOutput=False)
    sinIh_h = nc.declare_dram_parameter("sinIh", [F, L], FP16, isOutput=False)
    cosIl_h = nc.declare_dram_parameter("cosIl", [F, L], FP16, isOutput=False)
    sinIl_h = nc.declare_dram_parameter("sinIl", [F, L], FP16, isOutput=False)
    altF_h = nc.declare_dram_parameter("altF", [L, 1], FP16, isOutput=False)
    altI_h = nc.declare_dram_parameter("altI", [1, L], FP16, isOutput=False)
    out_h = nc.declare_dram_parameter("out", [BPC, L, H, E], F32, isOutput=True)

    with TileContext(nc) as tc:
        with (
            tc.tile_pool(name="tabs", bufs=1) as tabs,
            tc.tile_pool(name="io", bufs=1) as io,
            tc.tile_pool(name="spec", bufs=1) as spec,
            tc.tile_pool(name="qk32", bufs=1) as qk32p,
            tc.tile_pool(name="chunk", bufs=8) as chunkp,
            tc.tile_pool(name="corrp", bufs=1) as corrp,
            tc.tile_pool(name="small", bufs=1) as smallp,
            tc.tile_pool(name="oh", bufs=1) as ohp,
            tc.tile_pool(name="ost", bufs=2) as ostp,
            tc.tile_pool(name="psA", bufs=1, space="PSUM") as psA,
            tc.tile_pool(name="psB", bufs=4, space="PSUM") as psB,
        ):
            # ---- resident forward hi tables
            cosFt = tabs.tile([128, TC, F], FP16)
            sinFt = tabs.tile([128, TC, F], FP16)
            altF = tabs.tile([128, TC, 1], FP16)
            altI = tabs.tile([1, L], FP16)
            nc.sync.dma_start(cosFt[:], cosFh_h.rearrange("(c p) f -> p c f", p=128))
            nc.sync.dma_start(sinFt[:], sinFh_h.rearrange("(c p) f -> p c f", p=128))
            nc.sync.dma_start(altF[:], altF_h.rearrange("(c p) o -> p c o", p=128))
            nc.sync.dma_start(altI[:], altI_h[:])

            for g in range(groups):
                b = g // GPB
                h0 = (g % GPB) * HPG

                def load_in(tile, hbm):
                    nc.sync.dma_start(
                        tile[:],
                        hbm[b, :, h0:h0 + HPG, :].rearrange(
                            "(c p) h e -> p c (h e)", p=128
                        ),
                    )

                # ---- load q,k hi/lo tiles [t-part, tchunk, rows] rows=(h,e)
                qhT = io.tile([128, TC, R], FP16, tag="qhT")
                qlT = io.tile([128, TC, R], FP16, tag="qlT")
                khT = io.tile([128, TC, R], FP16, tag="khT")
                klT = io.tile([128, TC, R], FP16, tag="klT")
                load_in(qhT, qh_h)
                load_in(qlT, ql_h)
                load_in(khT, kh_h)
                load_in(klT, kl_h)

                # ---- forward DFTs of q,k (split precision) + P per k-chunk
                Prh = spec.tile([128, KC, R], FP16, tag="Prh")
                Prl = spec.tile([128, KC, R], FP16, tag="Prl")
                Pih = spec.tile([128, KC, R], FP16, tag="Pih")
                Pil = spec.tile([128, KC, R], FP16, tag="Pil")
                for kc in range(KC):
                    ks = slice(kc * 128, (kc + 1) * 128)
                    loC = chunkp.tile([128, TC, 128], FP16, tag="ch")
                    loS = chunkp.tile([128, TC, 128], FP16, tag="ch")
                    nc.sync.dma_start(
                        loC[:], cosFl_h[:, ks].rearrange("(c p) f -> p c f", p=128))
                    nc.sync.dma_start(
                        loS[:], sinFl_h[:, ks].rearrange("(c p) f -> p c f", p=128))
                    psf = psA.tile([128, 4 * 512], F32, tag="quad")
                    sl = [psf[:, j * 512:j * 512 + R] for j in range(4)]
                    for j, (hiT, loT, xh, xl) in enumerate((
                        (cosFt, loC, qhT, qlT),
                        (sinFt, loS, qhT, qlT),
                        (cosFt, loC, khT, klT),
                        (sinFt, loS, khT, klT),
                    )):
                        for tci in range(TC):
                            hp = hiT[:, tci, ks]
                            nc.tensor.matmul(sl[j], hp, xh[:, tci, :],
                                             start=(tci == 0), stop=False)
                            nc.tensor.matmul(sl[j], hp, xl[:, tci, :],
                                             start=False, stop=False)
                            nc.tensor.matmul(sl[j], loT[:, tci, :],
                                             xh[:, tci, :],
                                             start=False, stop=(tci == TC - 1))
                    # P = Q * conj(K) in f32, split to hi+lo fp16
                    qr = qk32p.tile([128, R], F32, tag="qr")
                    qi = qk32p.tile([128, R], F32, tag="qi")
                    kr = qk32p.tile([128, R], F32, tag="kr")
                    ki = qk32p.tile([128, R], F32, tag="ki")
                    p32 = qk32p.tile([128, R], F32, tag="p32")
                    t32 = qk32p.tile([128, R], F32, tag="t32")
                    nc.scalar.copy(qr[:], sl[0])
                    nc.scalar.copy(qi[:], sl[1])
                    nc.scalar.copy(kr[:], sl[2])
                    nc.scalar.copy(ki[:], sl[3])
                    nc.vector.tensor_mul(p32[:], qr[:], kr[:])
                    nc.vector.tensor_mul(t32[:], qi[:], ki[:])
                    nc.vector.tensor_add(p32[:], p32[:], t32[:])
                    nc.vector.tensor_copy(Prh[:, kc, :], p32[:])
                    nc.vector.tensor_sub(Prl[:, kc, :], p32[:], Prh[:, kc, :])
                    nc.vector.tensor_mul(p32[:], qi[:], kr[:])
                    nc.vector.tensor_mul(t32[:], qr[:], ki[:])
                    nc.vector.tensor_sub(p32[:], p32[:], t32[:])
                    nc.vector.tensor_copy(Pih[:, kc, :], p32[:])
                    nc.vector.tensor_sub(Pil[:, kc, :], p32[:], Pih[:, kc, :])
                # zero DC (band-pass)
                for P in (Prh, Prl, Pih, Pil):
                    nc.vector.memset(P[0:1, 0, :], 0.0)

                # ---- load v into the q slots (q consumed) and forward DFT
                vT = io.tile([128, TC, R], FP16, tag="qhT")
                load_in(vT, v_h)
                Vr = spec.tile([128, KC, R], FP16, tag="Vr")
                Vi = spec.tile([128, KC, R], FP16, tag="Vi")
                for kc in range(KC):
                    ks = slice(kc * 128, (kc + 1) * 128)
                    for tab, dst in ((cosFt, Vr), (sinFt, Vi)):
                        ps = psB.tile([128, R], F32, tag="bank")
                        for tci in range(TC):
                            nc.tensor.matmul(
                                ps[:], tab[:, tci, ks], vT[:, tci, :],
                                start=(tci == 0), stop=(tci == TC - 1),
                            )
                        nc.scalar.copy(dst[:, kc, :], ps[:])
                psn = psB.tile([128, R], F32, tag="bank")
                for tci in range(TC):
                    nc.tensor.matmul(
                        psn[0:1, :], altF[:, tci, :], vT[:, tci, :],
                        start=(tci == 0), stop=(tci == TC - 1),
                    )
                vn = smallp.tile([1, R], F32, tag="vn")
                nc.scalar.copy(vn[:], psn[0:1, :])

                # ---- per-rowchunk: corr, top-15, softmax, impulse train
                onehT = ohp.tile([128, TC, R], FP16, tag="onehT")
                for rh in range(2):
                    rs = slice(rh * 128, (rh + 1) * 128)
                    # corr[r,tau] = sum_k (Prh+Prl) cosI + (Pih+Pil) sinI
                    ps_corr = psA.tile([128, L], F32, tag="quad")
                    for kc in range(KC):
                        for Ph, Pl, hi_h, lo_h in (
                            (Prh, Prl, cosIh_h, cosIl_h),
                            (Pih, Pil, sinIh_h, sinIl_h),
                        ):
                            frst = (kc == 0 and Ph is Prh)
                            last = (kc == KC - 1 and Ph is Pih)
                            chh = chunkp.tile([128, L], FP16, tag="ch")
                            nc.sync.dma_start(
                                chh[:], hi_h[kc * 128:(kc + 1) * 128, :])
                            chl = chunkp.tile([128, L], FP16, tag="ch")
                            nc.sync.dma_start(
                                chl[:], lo_h[kc * 128:(kc + 1) * 128, :])
                            for t4 in range(4):
                                t4s = slice(t4 * 512, (t4 + 1) * 512)
                                nc.tensor.matmul(
                                    ps_corr[:, t4s], Ph[:, kc, rs], chh[:, t4s],
                                    start=frst, stop=False)
                                nc.tensor.matmul(
                                    ps_corr[:, t4s], Pl[:, kc, rs], chh[:, t4s],
                                    start=False, stop=False)
                                nc.tensor.matmul(
                                    ps_corr[:, t4s], Ph[:, kc, rs], chl[:, t4s],
                                    start=False, stop=last)
                    corr = corrp.tile([128, L], F32, tag="corr")
                    nc.scalar.copy(corr[:], ps_corr[:])

                    vals = smallp.tile([128, 16], F32, tag="vals")
                    idx = smallp.tile([128, 16], U16, tag="idx")
                    nc.vector.max(vals[:, 0:8], corr[:])
                    nc.vector.max_index(idx[:, 0:8], vals[:, 0:8], corr[:])
                    nc.vector.match_replace(corr[:], vals[:, 0:8], corr[:], NEG)
                    nc.vector.max(vals[:, 8:16], corr[:])
                    nc.vector.max_index(idx[:, 8:16], vals[:, 8:16], corr[:])

                    # softmax over the 15 kept lags (16th gets weight 0)
                    negm = smallp.tile([128, 1], F32, tag="negm")
                    ew = smallp.tile([128, 16], F32, tag="ew")
                    ssum = smallp.tile([128, 1], F32, tag="ssum")
                    rsum = smallp.tile([128, 1], F32, tag="rsum")
                    w16 = smallp.tile([128, 16], FP16, tag="w16")
                    nc.vector.tensor_scalar(
                        negm[:], vals[:, 0:1], -1.0, None, op0=MUL)
                    nc.scalar.activation(
                        ew[:, 0:15], vals[:, 0:15],
                        mybir.ActivationFunctionType.Exp,
                        bias=negm[:], scale=1.0)
                    nc.vector.memset(ew[:, 15:16], 0.0)
                    nc.vector.reduce_sum(ssum[:], ew[:, 0:15], axis=AX)
                    nc.vector.reciprocal(rsum[:], ssum[:])
                    nc.vector.tensor_scalar(w16[:], ew[:], rsum[:], None, op0=MUL)

                    # scatter indices: lo = d (d<1024) else neg ; hi = d-1024
                    idxf = smallp.tile([128, 16], F32, tag="idxf")
                    tge = smallp.tile([128, 16], F32, tag="tge")
                    slo = smallp.tile([128, 16], I16, tag="slo")
                    shi = smallp.tile([128, 16], I16, tag="shi")
                    nc.vector.tensor_copy(idxf[:], idx[:])
                    nc.vector.tensor_scalar(
                        tge[:], idxf[:], 1024.0, None,
                        op0=mybir.AluOpType.is_ge)
                    nc.vector.scalar_tensor_tensor(
                        slo[:], tge[:], -2048.0, idxf[:],
                        op0=MUL, op1=mybir.AluOpType.add)
                    nc.vector.tensor_scalar(
                        shi[:], idxf[:], -1024.0, None,
                        op0=mybir.AluOpType.add)

                    oneh = ohp.tile([128, L], FP16, tag="oneh")
                    nc.gpsimd.local_scatter(
                        oneh[:, 0:1024], w16[:], slo[:],
                        channels=128, num_elems=1024, num_idxs=16)
                    nc.gpsimd.local_scatter(
                        oneh[:, 1024:2048], w16[:], shi[:],
                        channels=128, num_elems=1024, num_idxs=16)

                    # transpose impulse train to [t, rows]
                    nc.sync.dma_start_transpose(
                        onehT[:, :, rs], oneh[:])

                # ---- S = DFT(s) and S Nyquist
                Sr = spec.tile([128, KC, R], FP16, tag="Pil")
                Si = spec.tile([128, KC, R], FP16, tag="Si")
                sn = smallp.tile([1, R], F32, tag="sn")
                for kc in range(KC):
                    ks = slice(kc * 128, (kc + 1) * 128)
                    for tab, dst in ((cosFt, Sr), (sinFt, Si)):
                        ps = psB.tile([128, R], F32, tag="bank")
                        for tci in range(TC):
                            nc.tensor.matmul(
                                ps[:], tab[:, tci, ks], onehT[:, tci, :],
                                start=(tci == 0), stop=(tci == TC - 1))
                        nc.scalar.copy(dst[:, kc, :], ps[:])
                psn2 = psB.tile([128, R], F32, tag="bank")
                for tci in range(TC):
                    nc.tensor.matmul(
                        psn2[0:1, :], altF[:, tci, :], onehT[:, tci, :],
                        start=(tci == 0), stop=(tci == TC - 1))
                nc.scalar.copy(sn[:], psn2[0:1, :])
                anyq = smallp.tile([1, R], FP16, tag="anyq")
                nc.vector.tensor_mul(anyq[:], vn[:], sn[:])

                # ---- A = V * conj(S)   (DC kept: exact mean via k=0 tables)
                Ar = spec.tile([128, KC, R], FP16, tag="Prh")
                Ai = spec.tile([128, KC, R], FP16, tag="Pih")
                tmp2 = spec.tile([128, KC, R], FP16, tag="Prl")
                nc.vector.tensor_mul(Ar[:], Vr[:], Sr[:])
                nc.vector.tensor_mul(tmp2[:], Vi[:], Si[:])
                nc.vector.tensor_add(Ar[:], Ar[:], tmp2[:])
                nc.vector.tensor_mul(Ai[:], Vi[:], Sr[:])
                nc.vector.tensor_mul(tmp2[:], Vr[:], Si[:])
                nc.vector.tensor_sub(Ai[:], Ai[:], tmp2[:])

                # ---- agg[r,tau] = sum_k Ar cosI + Ai sinI + anyq[r] altI[tau]
                for rh in range(2):
                    rs = slice(rh * 128, (rh + 1) * 128)
                    ps_agg = psA.tile([128, L], F32, tag="quad")
                    for kc in range(KC):
                        for comp, (A, tab_h) in enumerate(
                            ((Ar, cosIh_h), (Ai, sinIh_h))
                        ):
                            ch = chunkp.tile([128, L], FP16, tag="ch")
                            nc.sync.dma_start(
                                ch[:], tab_h[kc * 128:(kc + 1) * 128, :])
                            for t4 in range(4):
                                t4s = slice(t4 * 512, (t4 + 1) * 512)
                                nc.tensor.matmul(
                                    ps_agg[:, t4s], A[:, kc, rs], ch[:, t4s],
                                    start=(kc == 0 and comp == 0), stop=False)
                    for t4 in range(4):
                        t4s = slice(t4 * 512, (t4 + 1) * 512)
                        nc.tensor.matmul(
                            ps_agg[:, t4s], anyq[0:1, rs], altI[0:1, t4s],
                            start=False, stop=True)
                    # fp16 staging -> xbar transpose -> f32 -> HBM
                    a16 = ohp.tile([128, L], FP16, tag="oneh")
                    nc.scalar.copy(a16[:], ps_agg[:])
                    aT = ohp.tile([128, TC, 128], FP16, tag="aT")
                    nc.sync.dma_start_transpose(aT[:], a16[:])
                    for tci in range(TC):
                        ost = ostp.tile([128, 128], F32, tag="ost")
                        nc.vector.tensor_copy(ost[:], aT[:, tci, :])
                        nc.sync.dma_start(
                            out_h[
                                b, tci * 128:(tci + 1) * 128,
                                h0 + 2 * rh:h0 + 2 * rh + 2, :
                            ].rearrange("t h e -> t (h e)"),
                            ost[:])
    nc.compile()
    return nc


# ---------------------------------------------------------------- host tables

def _hilo(x):
    h = x.astype(np.float16)
    l = (x - h.astype(np.float64)).astype(np.float16)
    return h, l


def _make_tables():
    t = np.arange(L, dtype=np.float64)
    k = np.arange(F, dtype=np.float64)
    ang = 2.0 * np.pi / L
    tk = np.outer(t, k)
    cosF = np.cos(ang * tk)
    sinF = -np.sin(ang * tk)
    kt = np.outer(k, t)
    cosI = (2.0 / L) * np.cos(ang * kt)
    sinI = -(2.0 / L) * np.sin(ang * kt)
    cosI[0, :] = 1.0 / L
    sinI[0, :] = 0.0
    altF = ((-1.0) ** t)[:, None]
    altI = (((-1.0) ** t) / L)[None, :]
    cosFh, cosFl = _hilo(cosF)
    sinFh, sinFl = _hilo(sinF)
    cosIh, cosIl = _hilo(cosI)
    sinIh, sinIl = _hilo(sinI)
    return {
        "cosFh": cosFh, "sinFh": sinFh, "cosFl": cosFl, "sinFl": sinFl,
        "cosIh": cosIh, "sinIh": sinIh, "cosIl": cosIl, "sinIl": sinIl,
        "altF": altF.astype(np.float16), "altI": altI.astype(np.float16),
    }


TABLE_NAMES = ["cosFh", "sinFh", "cosFl", "sinFl",
               "cosIh", "sinIh", "cosIl", "sinIl", "altF", "altI"]
IN_NAMES = ("qh", "ql", "kh", "kl", "v", *TABLE_NAMES, "out")


# ---------------------------------------------------------------- host runner

def _get_runner():
    if "run" in _state:
        return _state["run"]

    import jax
    from jax.sharding import Mesh, PartitionSpec, NamedSharding
    try:
        from jax.experimental.shard_map import shard_map
    except ImportError:
        from jax.shard_map import shard_map
    from concourse import bass2jax

    bass2jax.install_neuronx_cc_hook()

    nc = build_nc()
    tables = _make_tables()

    devices = jax.devices()[:NCORES]
    mesh = Mesh(np.asarray(devices), ("core",))
    shard = NamedSharding(mesh, PartitionSpec("core"))
    repl = NamedSharding(mesh, PartitionSpec())

    out_avals = (jax.core.ShapedArray((BPC, L, H, E), np.float32),)

    pid_name = nc.partition_id_tensor.name if nc.partition_id_tensor else None
    body_in_names = IN_NAMES + ((pid_name,) if pid_name else ())

    def _body(*args):
        operands = list(args)
        if pid_name:
            operands.append(bass2jax.partition_id_tensor())
        outs = bass2jax._bass_exec_p.bind(
            *operands,
            out_avals=out_avals,
            in_names=body_in_names,
            out_names=("out",),
            lowering_input_output_aliases=(),
            sim_require_finite=False,
            sim_require_nnan=False,
            nc=nc,
        )
        return tuple(outs)

    P = PartitionSpec
    n_tab = len(TABLE_NAMES)
    sharded = jax.jit(
        shard_map(
            _body, mesh=mesh,
            in_specs=(P("core"),) * 5 + (P(),) * n_tab + (P("core"),),
            out_specs=(P("core"),),
            check_rep=False,
        ),
        keep_unused=True,
    )

    tab_dev = [jax.device_put(tables[n], repl) for n in TABLE_NAMES]
    zout = jax.device_put(np.zeros((B, L, H, E), np.float32), shard)

    def run(qh, ql, kh, kl, v16):
        (out,) = sharded(qh, ql, kh, kl, v16, *tab_dev, zout)
        return np.asarray(out)

    _state["run"] = run
    return run


def _split_inputs(queries, keys, values):
    """fp16 hi/lo of q,k and fp16 of v, parallel over batch."""
    from concurrent.futures import ThreadPoolExecutor
    q = np.asarray(queries)
    k = np.asarray(keys)
    v = np.asarray(values)
    qh = np.empty(q.shape, np.float16); ql = np.empty(q.shape, np.float16)
    kh = np.empty(k.shape, np.float16); kl = np.empty(k.shape, np.float16)
    vh = np.empty(v.shape, np.float16)

    def do(i):
        qh[i] = q[i]
        ql[i] = q[i] - qh[i].astype(np.float32)
        kh[i] = k[i]
        kl[i] = k[i] - kh[i].astype(np.float32)
        vh[i] = v[i]

    with ThreadPoolExecutor(max_workers=16) as ex:
        list(ex.map(do, range(q.shape[0])))
    return qh, ql, kh, kl, vh


def kernel(queries, keys, values, attn_mask=None, **_ignored):
    run = _get_runner()
    qh, ql, kh, kl, vh = _split_inputs(queries, keys, values)
    return run(qh, ql, kh, kl, vh)


if __name__ == "__main__":
    rng = np.random.default_rng(0)
    out = kernel(
        queries=rng.standard_normal((B, L, H, E), dtype=np.float32),
        keys=rng.standard_normal((B, L, H, E), dtype=np.float32),
        values=rng.standard_normal((B, L, H, E), dtype=np.float32),
        attn_mask=np.zeros((1,), dtype=bool),
    )
    print(out.shape, out.dtype, float(np.abs(out).max()))
